# revision 3
# baseline (speedup 1.0000x reference)
"""CausalAttention (B=4, T=2048, C=1024, H=16, D=64) on 8 TRN2 NeuronCores.

Sharding: core c -> (batch b = c//2, head-group hg = c%2 covering heads
hg*8..hg*8+7).  Each core computes QKV for its batch restricted to its 8
heads, causal attention, and the output projection over the AllGathered
at for half the rows of each q-chunk.

Device algorithm (per core, bf16 matmuls):
  QKV units: qkT[j,t] = Wqk^T x^T   (Q,K kept transposed: [channels, T])
             V[t,j]   = x Wv        (ones column per head via memset)
  attention per q-chunk of 512, per head-pair hp:
           sT[k,q] = K_h^T q-block (scores transposed; causal blocks only)
           expT = exp(0.125*sT)    (ACT, PSUM->SBUF bf16)
           diag slabs: expT *= keep-mask  (one DVE op for both heads)
           out'[d,q] (+ sumexp in row 64) = V'_h^T @ expT (accum over k)
           at[c,q] = out'[0:64]/sumexp  (recip + partition_broadcast + mul)
  exchange: per chunk TWO pairwise AllGathers (heads 0-3 after hp1,
           heads 4-7 after hp3) via DRAM staging, overlapping compute
  proj:    at^T Wproj + bias for my 256 rows per chunk (agt prefetched)

Schedule: QKV units, agt prefetches, and proj pieces are interleaved into
the attention chunks at head-pair granularity so the PE queue never runs
dry (PE p-state: ~1.2GHz until ~3us of continuous execution, then 2.4GHz
-- every bubble halves throughput for the next 3us).  Input DMAs are
issued on one queue in consumption order so early transfers are not
bandwidth-starved by later ones.
"""
import ml_dtypes
import numpy as np

import concourse.bass as bass
import concourse.tile as tile
from concourse import bacc, mybir
from concourse.bass_utils import run_bass_kernel_spmd

F32 = mybir.dt.float32
AF = mybir.ActivationFunctionType

B, T, C = 4, 2048, 1024
H, D = 16, 64
HL = 8           # heads per core
CL = HL * D      # local channels (512)
CDT = mybir.dt.bfloat16  # matmul compute dtype
QC = 512         # q-chunk width
NQC = T // QC    # 4
KT = 128         # k-tile
N5 = 512         # matmul free-dim / PSUM bank width (fp32)


def _build():
    nc = bacc.Bacc("TRN2", target_bir_lowering=False, debug=False, num_devices=8)

    xT = nc.dram_tensor("xT", [8, 128, T], CDT, kind="ExternalInput").ap()
    wqk = nc.dram_tensor("wqk", [8, 128, 1024], CDT, kind="ExternalInput").ap()
    wv = nc.dram_tensor("wv", [8, 128, CL], CDT, kind="ExternalInput").ap()
    wproj = nc.dram_tensor("wproj", [8, 128, C], CDT, kind="ExternalInput").ap()
    bias2 = nc.dram_tensor("bias2", [1, C], CDT, kind="ExternalInput").ap()
    ones_r = nc.dram_tensor("ones_r", [1, 128], CDT, kind="ExternalInput").ap()
    keep2 = nc.dram_tensor("keep2", [128, 2 * 128], CDT, kind="ExternalInput").ap()
    out = nc.dram_tensor("out", [T // 2, C], F32, kind="ExternalOutput").ap()

    with tile.TileContext(nc) as tc:
        _emit(nc, tc, xT, wqk, wv, wproj, bias2, ones_r, keep2, out)

    nc.compile()
    return nc


def _emit(nc, tc, xT, wqk, wv, wproj, bias2, ones_r, keep2, out):
    with (
        tc.tile_pool(name="persist", bufs=1) as pp,
        tc.tile_pool(name="xtp", bufs=16) as xtp,
        tc.tile_pool(name="ps_s", bufs=2, space="PSUM") as ps_s,
        tc.tile_pool(name="ps_av", bufs=4, space="PSUM") as ps_av,
        tc.tile_pool(name="expp", bufs=4) as expp,
        tc.tile_pool(name="atp", bufs=2) as atp,
        tc.tile_pool(name="nrm", bufs=4) as nrm,
        tc.tile_pool(name="stg", bufs=2) as stg,
        tc.tile_pool(name="drp", bufs=16, space="DRAM") as drp,
    ):
        # qkT[jt]: channels 128*jt..128*jt+127 (j<512: Q; j>=512: K), [128, T]
        qkT = [pp.tile([128, T], CDT, name=f"qkT{j}") for j in range(8)]
        # VV[tb]: [128 t, HL heads, D+1] - col D is the ones column (sumexp)
        VV = [pp.tile([128, HL, D + 1], CDT, name=f"VV{t}") for t in range(T // 128)]
        wqk_t = [pp.tile([128, 1024], CDT, name=f"wqk{i}") for i in range(8)]
        wv_t = [pp.tile([128, CL], CDT, name=f"wv{i}") for i in range(8)]
        wproj_t8 = [pp.tile([128, C], CDT, name=f"wproj{i}") for i in range(8)]
        bias_t = pp.tile([1, C], CDT, name="bias_t")
        ones_t = pp.tile([1, 128], CDT, name="ones_t")
        keep2_t = pp.tile([128, 2, 128], CDT, name="keep2_t")

        # ---- input loads: ONE issue queue, strict consumption order so the
        # critical first 4MB (xt tch0 + wqk) is never bandwidth-starved by
        # later transfers.  ~0.65us issue each, 8 HW rings round-robin.
        xt_all = {0: [], 1: []}
        for cb in range(8):
            x_t = xtp.tile([128, 1024], CDT, tag="xt", name=f"xt0_{cb}")
            nc.sync.dma_start(out=x_t, in_=xT[cb, :, 0:1024])
            xt_all[0].append(x_t)
            nc.sync.dma_start(out=wqk_t[cb], in_=wqk[cb])
        for i in range(8):
            nc.sync.dma_start(out=wv_t[i], in_=wv[i])
        nc.sync.dma_start(out=keep2_t, in_=keep2.rearrange("p (a b) -> p a b", a=2))
        nc.sync.dma_start(out=bias_t, in_=bias2)
        nc.sync.dma_start(out=ones_t, in_=ones_r)
        for cb in range(8):
            x_t = xtp.tile([128, 1024], CDT, tag="xt", name=f"xt1_{cb}")
            nc.sync.dma_start(out=x_t, in_=xT[cb, :, 1024:2048])
            xt_all[1].append(x_t)
        for i in range(8):
            nc.sync.dma_start(out=wproj_t8[i], in_=wproj[i])
        # ones columns of VV: constant, no DMA ring needed
        for gtb in range(16):
            nc.gpsimd.memset(VV[gtb][:, :, D:D + 1], 1.0)

        at_all, ags = {}, {}
        with tc.tile_critical():
            rid = nc.sync.partition_id()
            rankoff = (rid % 2) * (QC // 2)

        # ---- QKV phase units (emitted piecemeal, interleaved into attention)
        def qk_unit(tch, jt, s5):
            t0 = tch * 1024
            xt = xt_all[tch]
            ps = ps_s.tile([128, N5], F32, tag="s", name=f"pqk{tch}{jt}{s5}")
            for cb in range(8):
                nc.tensor.matmul(
                    ps, wqk_t[cb][:, jt * 128:(jt + 1) * 128],
                    xt[cb][:, s5 * N5:(s5 + 1) * N5],
                    start=(cb == 0), stop=(cb == 7))
            nc.vector.tensor_copy(
                qkT[jt][:, t0 + s5 * N5: t0 + (s5 + 1) * N5], ps)

        def v_unit(gtb):
            tch, tb = gtb // 8, gtb % 8
            xt = xt_all[tch]
            ps = ps_s.tile([128, CL], F32, tag="s", name=f"pv{gtb}")
            for cb in range(8):
                nc.tensor.matmul(
                    ps, xt[cb][:, tb * 128:(tb + 1) * 128], wv_t[cb],
                    start=(cb == 0), stop=(cb == 7))
            nc.vector.tensor_copy(
                VV[gtb][:, :, 0:D],
                ps.rearrange("p (h d) -> p h d", h=HL))

        def exchange_half(pqc, half):
            """AllGather heads half*4..half*4+3 (at tiles ci=2*half,2*half+1)."""
            pat = at_all[pqc]
            ad = drp.tile([256, QC], CDT, tag=f"atd{half}", name=f"atd{pqc}_{half}")
            for k in range(2):
                nc.sync.dma_start(
                    out=ad[k * 128:(k + 1) * 128, :], in_=pat[2 * half + k])
            ag = drp.tile([2, 256, QC], CDT, tag=f"atg{half}",
                          name=f"atg{pqc}_{half}")
            nc.gpsimd.collective_compute(
                "AllGather", mybir.AluOpType.bypass,
                replica_groups=[[0, 1], [2, 3], [4, 5], [6, 7]],
                ins=[ad[:]], outs=[ag[:]])
            ags[(pqc, half)] = ag

        # channel block ci8 (128 rows of the 1024-ch at) -> (ag half, rank, k)
        # ag_a rows: [hg0 ch 0-255 ; hg1 ch 512-767], ag_b: [256-511 ; 768-1023]
        def agt_prefetch(pqc):
            agt = [stg.tile([128, QC // 2], CDT, tag=f"agt{ci8}",
                            name=f"agt{pqc}_{ci8}")
                   for ci8 in range(8)]
            for ci8 in range(8):
                rank, half, k = ci8 // 4, (ci8 % 4) // 2, ci8 % 2
                ag = ags[(pqc, half)]
                nc.sync.dma_start(
                    out=agt[ci8],
                    in_=ag[rank, k * 128:(k + 1) * 128,
                           bass.ds(rankoff, QC // 2)])
            return agt

        def proj_piece(pqc, agt):
            """proj + output for MY 256 rows (rank offset) of chunk pqc."""
            for tt in range(QC // 256):
                st = stg.tile([128, C], F32, tag="stage", name=f"stg{pqc}_{tt}")
                for jc in range(2):
                    pp_ps = ps_s.tile([128, N5], F32, tag="s",
                                      name=f"pp{pqc}_{tt}_{jc}")
                    for ci8 in range(8):
                        nc.tensor.matmul(
                            pp_ps, agt[ci8][:, tt * 128:(tt + 1) * 128],
                            wproj_t8[ci8][:, jc * N5:(jc + 1) * N5],
                            start=(ci8 == 0), stop=False)
                    nc.tensor.matmul(
                        pp_ps, ones_t, bias_t[0:1, jc * N5:(jc + 1) * N5],
                        start=False, stop=True)
                    nc.vector.tensor_copy(st[:, jc * N5:(jc + 1) * N5], pp_ps)
                r0 = pqc * (QC // 2) + tt * 128
                # two half-row DMAs so the 512KB write spreads over 2 rings
                nc.sync.dma_start(out=out[r0:r0 + 128, 0:N5],
                                  in_=st[:, 0:N5])
                nc.sync.dma_start(out=out[r0:r0 + 128, N5:C],
                                  in_=st[:, N5:C])

        def emit_attention(qc, interleave):
            """attention for chunk qc; `interleave` maps hp -> list of fns."""
            q0 = qc * QC
            nkt = (q0 + QC) // KT
            at = [atp.tile([128, QC], CDT, tag=f"at{ci}", name=f"at{qc}_{ci}")
                  for ci in range(4)]
            at_all[qc] = at
            for hp in range(HL // 2):
                heads = (2 * hp, 2 * hp + 1)
                av = {h: ps_av.tile([D + 1, N5], F32, tag="av",
                                    name=f"av{qc}_{h}")
                      for h in heads}
                exps = {}

                def emit_scores(kt):
                    k0 = kt * KT
                    est = max(0, k0 - q0)
                    # pair-shared score tile: head h at free half h%2
                    sp = ps_s.tile([128, 2, N5], F32, tag="s",
                                   name=f"s{qc}_{hp}_{kt}")
                    for h in heads:
                        roff = (h % 2) * D
                        nc.tensor.matmul(
                            sp[:, h % 2, est:N5],
                            qkT[4 + h // 2][roff:roff + D, k0:k0 + KT],
                            qkT[h // 2][roff:roff + D, q0 + est:q0 + QC],
                            start=True, stop=True)
                    ex = expp.tile([128, 2, N5], CDT, tag="exp",
                                   name=f"ex{qc}_{hp}_{kt}")
                    nc.scalar.activation(
                        ex[:, :, est:N5], sp[:, :, est:N5],
                        AF.Exp, scale=0.125)
                    if k0 >= q0:  # zero masked part of the diagonal slab
                        nc.vector.tensor_mul(
                            ex[:, :, est:est + KT],
                            ex[:, :, est:est + KT], keep2_t)
                    exps[kt] = ex

                def emit_attnv(kt):
                    k0 = kt * KT
                    cst = max(0, k0 - q0)
                    ex = exps.pop(kt)
                    for h in heads:
                        nc.tensor.matmul(
                            av[h][:, cst:N5], VV[kt][:, h, :],
                            ex[:, h % 2, cst:N5],
                            start=(kt == 0), stop=(kt == nkt - 1))

                emit_scores(0)
                for kt in range(1, nkt):
                    emit_scores(kt)
                    emit_attnv(kt - 1)
                emit_attnv(nkt - 1)

                for fn in interleave.get(hp, []):
                    fn()

                for h in heads:
                    roff = (h % 2) * D
                    a = av[h]
                    # custom-DVE/gpsimd ops need partition-0-aligned inputs;
                    # plain DVE copy handles the 64->0 shift (PSUM read)
                    rc0 = nrm.tile([1, N5], F32, tag="rc0",
                                   name=f"rc0{qc}_{h}")
                    nc.vector.tensor_copy(rc0, a[D:D + 1, :])
                    rc = nrm.tile([1, N5], F32, tag="rc", name=f"rc{qc}_{h}")
                    nc.vector.reciprocal_approx_fast(out=rc, in_=rc0)
                    rb = nrm.tile([D, N5], F32, tag="rb", name=f"rb{qc}_{h}")
                    nc.gpsimd.partition_broadcast(rb, rc)
                    nc.vector.tensor_mul(at[h // 2][roff:roff + D, :],
                                         a[0:D, :], rb)
                if hp == 1:
                    exchange_half(qc, 0)
                elif hp == 3:
                    exchange_half(qc, 1)

        # ---- schedule -----------------------------------------------------
        # bootstrap: exactly what qc0-hp0 needs (heads 0,1 + V tiles 0-3),
        # everything else flows just-in-time through the hp interleave slots.
        def U(tch, jt, s5):
            return lambda: qk_unit(tch, jt, s5)

        def Vu(gtb):
            return lambda: v_unit(gtb)

        qk_unit(0, 0, 0)
        qk_unit(0, 4, 0)
        for tb in range(4):
            v_unit(tb)

        prj = {}

        def PF(pqc):
            def f():
                prj[pqc] = agt_prefetch(pqc)
            return f

        def PJ(pqc):
            return lambda: proj_piece(pqc, prj.pop(pqc))

        emit_attention(0, {
            0: [U(0, 1, 0), U(0, 5, 0), Vu(4)],
            1: [U(0, 2, 0), U(0, 6, 0), Vu(5)],
            2: [U(0, 3, 0), U(0, 7, 0), Vu(6)],
            3: [U(0, 0, 1), U(0, 4, 1), Vu(7)],
        })
        emit_attention(1, {
            0: [U(0, 1, 1), U(0, 5, 1), Vu(8)],
            1: [U(0, 2, 1), U(0, 6, 1), Vu(9)],
            2: [U(0, 3, 1), U(0, 7, 1), Vu(10)],
            3: [U(1, 0, 0), U(1, 4, 0), Vu(11)],
        })
        emit_attention(2, {
            0: [U(1, 1, 0), U(1, 5, 0), Vu(12), PF(0)],
            1: [U(1, 2, 0), U(1, 6, 0), Vu(13), PJ(0)],
            2: [U(1, 3, 0), U(1, 7, 0), Vu(14), PF(1)],
            3: [U(1, 0, 1), U(1, 4, 1), Vu(15), PJ(1)],
        })
        emit_attention(3, {
            0: [U(1, 1, 1), U(1, 5, 1)],
            1: [U(1, 2, 1), U(1, 6, 1), PF(2)],
            2: [U(1, 3, 1), U(1, 7, 1), PJ(2)],
        })
        prj[3] = agt_prefetch(3)
        proj_piece(3, prj.pop(3))


def _prepare_in_maps(x, Wqkv, Wproj, bproj):
    x = np.asarray(x, dtype=np.float32)
    Wqkv = np.asarray(Wqkv, dtype=np.float32)
    Wproj = np.asarray(Wproj, dtype=np.float32)
    bproj = np.asarray(bproj, dtype=np.float32)

    # triangular keep-mask slab: 1 where q >= k (kept), 0 where masked;
    # duplicated side by side for the two heads of a pair
    k_i = np.arange(128)[:, None]
    q_i = np.arange(128)[None, :]
    keep = np.where(q_i >= k_i, np.float32(1.0), np.float32(0.0))
    keep2 = np.ascontiguousarray(
        np.concatenate([keep, keep], axis=1), dtype=np.float32)

    ones_r = np.ones((1, 128), dtype=np.float32)

    in_maps = []
    for core in range(8):
        b, hg = core // 2, core % 2
        xT = np.ascontiguousarray(x[b].T).reshape(8, 128, T)
        wq = Wqkv[:, hg * CL:(hg + 1) * CL]
        wk = Wqkv[:, C + hg * CL: C + (hg + 1) * CL]
        wv_ = Wqkv[:, 2 * C + hg * CL: 2 * C + (hg + 1) * CL]
        wqk = np.ascontiguousarray(
            np.concatenate([wq, wk], axis=1)).reshape(8, 128, 1024)
        wv = np.ascontiguousarray(wv_).reshape(8, 128, CL)
        wp = np.ascontiguousarray(Wproj).reshape(8, 128, C)
        bf = ml_dtypes.bfloat16
        in_maps.append({
            "xT": xT.astype(bf), "wqk": wqk.astype(bf), "wv": wv.astype(bf),
            "wproj": wp.astype(bf),
            "bias2": bproj.reshape(1, C).astype(bf),
            "ones_r": ones_r.astype(bf),
            "keep2": keep2.astype(bf),
        })
    return in_maps


def _assemble(results):
    full = np.empty((B, T, C), dtype=np.float32)
    for core in range(8):
        b, r = core // 2, core % 2
        o = results[core]["out"]  # [1024, 1024]
        for qc in range(NQC):
            g0 = qc * QC + r * (QC // 2)
            full[b, g0:g0 + QC // 2] = \
                o[qc * (QC // 2):(qc + 1) * (QC // 2)]
    return full


_NC_CACHE = None


def kernel(x, Wqkv, Wproj, bproj):
    global _NC_CACHE
    if _NC_CACHE is None:
        _NC_CACHE = _build()
    in_maps = _prepare_in_maps(x, Wqkv, Wproj, bproj)
    res = run_bass_kernel_spmd(_NC_CACHE, in_maps, list(range(8)))
    return _assemble(res.results)


# revision 8
# speedup vs baseline: 1.2295x; 1.2295x over previous
"""CausalAttention (B=4, T=2048, C=1024, H=16, D=64) on 8 TRN2 NeuronCores.

Sharding: core c -> (batch b = c//2, head-group hg = c%2 covering heads
hg*8..hg*8+7).  Each core computes QKV for its batch restricted to its 8
heads, causal attention, and the output projection over the AllGathered
at for half the rows of each q-chunk.

Device algorithm (per core, bf16 matmuls):
  QKV units: qkT[j,t] = Wqk^T x^T   (Q,K kept transposed: [channels, T])
             V[t,j]   = x Wv        (ones column per head via memset)
  attention per q-chunk of 512, per head-pair hp:
           sT[k,q] = K_h^T q-block (scores transposed; causal blocks only;
                     the two heads run in concurrent 64-row PE groups)
           expT = exp(0.125*sT)    (ACT, PSUM->SBUF bf16)
           diag slabs: expT *= keep-mask  (one DVE op for both heads)
           out'[d,q] (+ sumexp in row 64) = V'_h^T @ expT (accum over k)
           at[c,q] = out'[0:64]/sumexp  (recip + partition_broadcast + mul)
  exchange: per chunk TWO pairwise AllGathers (heads 0-3 after hp1,
           heads 4-7 after hp3) via DRAM staging, overlapping compute
  proj:    at^T Wproj + bias for my 256 rows per chunk (agt prefetched);
           the last chunk's proj is split so its first-half-channel
           matmuls run during the final AllGather.

Scheduling: the attention inner loop is exp-bound (ACT ~1.3us per k-tile
vs ~0.9us of PE work), and the TRN2 PE drops to 1.2GHz after any bubble
(re-ramping to 2.4GHz only after ~3us of continuous work).  So QKV units
are injected INSIDE the k-tile loops every `cadence` iterations to keep
the PE saturated, ordered so every unit lands before its first consumer.
PSUM->SBUF copies go to the GpSimd engine to keep the DVE queue short.
Input DMAs are issued on one queue in consumption order.
"""
import ml_dtypes
import numpy as np

import concourse.bass as bass
import concourse.tile as tile
from concourse import bacc, mybir
from concourse.bass_utils import run_bass_kernel_spmd

F32 = mybir.dt.float32
AF = mybir.ActivationFunctionType

B, T, C = 4, 2048, 1024
H, D = 16, 64
HL = 8           # heads per core
CL = HL * D      # local channels (512)
CDT = mybir.dt.bfloat16  # matmul compute dtype
QC = 512         # q-chunk width
NQC = T // QC    # 4
KT = 128         # k-tile
N5 = 512         # matmul free-dim / PSUM bank width (fp32)


def _build():
    nc = bacc.Bacc("TRN2", target_bir_lowering=False, debug=False, num_devices=8)

    xT = nc.dram_tensor("xT", [8, 128, T], CDT, kind="ExternalInput").ap()
    wqk = nc.dram_tensor("wqk", [8, 128, 1024], CDT, kind="ExternalInput").ap()
    wv = nc.dram_tensor("wv", [8, 128, CL], CDT, kind="ExternalInput").ap()
    wproj = nc.dram_tensor("wproj", [8, 128, C], CDT, kind="ExternalInput").ap()
    bias2 = nc.dram_tensor("bias2", [1, C], CDT, kind="ExternalInput").ap()
    ones_r = nc.dram_tensor("ones_r", [1, 128], CDT, kind="ExternalInput").ap()
    keep2 = nc.dram_tensor("keep2", [128, 2 * 128], CDT, kind="ExternalInput").ap()
    out = nc.dram_tensor("out", [T // 2, C], F32, kind="ExternalOutput").ap()

    with tile.TileContext(nc) as tc:
        _emit(nc, tc, xT, wqk, wv, wproj, bias2, ones_r, keep2, out)

    nc.compile()
    return nc


def _emit(nc, tc, xT, wqk, wv, wproj, bias2, ones_r, keep2, out):
    with (
        tc.tile_pool(name="persist", bufs=1) as pp,
        tc.tile_pool(name="xtp", bufs=16) as xtp,
        tc.tile_pool(name="ps_s", bufs=2, space="PSUM") as ps_s,
        tc.tile_pool(name="ps_av", bufs=4, space="PSUM") as ps_av,
        tc.tile_pool(name="expp", bufs=4) as expp,
        tc.tile_pool(name="atp", bufs=2) as atp,
        tc.tile_pool(name="nrm", bufs=4) as nrm,
        tc.tile_pool(name="stg", bufs=4) as stg,
        tc.tile_pool(name="drp", bufs=16, space="DRAM") as drp,
    ):
        # qkT[jt]: channels 128*jt..128*jt+127 (j<512: Q; j>=512: K), [128, T]
        qkT = [pp.tile([128, T], CDT, name=f"qkT{j}") for j in range(8)]
        # VV[tb]: [128 t, HL heads, D+1] - col D is the ones column (sumexp)
        VV = [pp.tile([128, HL, D + 1], CDT, name=f"VV{t}") for t in range(T // 128)]
        wqk_t = [pp.tile([128, 1024], CDT, name=f"wqk{i}") for i in range(8)]
        wv_t = [pp.tile([128, CL], CDT, name=f"wv{i}") for i in range(8)]
        wproj_t8 = [pp.tile([128, C], CDT, name=f"wproj{i}") for i in range(8)]
        bias_t = pp.tile([1, C], CDT, name="bias_t")
        ones_t = pp.tile([1, 128], CDT, name="ones_t")
        keep2_t = pp.tile([128, 2, 128], CDT, name="keep2_t")

        # ---- input loads: ONE issue queue, strict consumption order so the
        # critical first 4MB (xt tch0 + wqk) is never bandwidth-starved.
        xt_all = {0: [], 1: []}
        for cb in range(8):
            x_t = xtp.tile([128, 1024], CDT, tag="xt", name=f"xt0_{cb}")
            nc.sync.dma_start(out=x_t, in_=xT[cb, :, 0:1024])
            xt_all[0].append(x_t)
            nc.sync.dma_start(out=wqk_t[cb], in_=wqk[cb])
        for i in range(8):
            nc.sync.dma_start(out=wv_t[i], in_=wv[i])
        nc.sync.dma_start(out=keep2_t, in_=keep2.rearrange("p (a b) -> p a b", a=2))
        nc.sync.dma_start(out=bias_t, in_=bias2)
        nc.sync.dma_start(out=ones_t, in_=ones_r)
        for cb in range(8):
            x_t = xtp.tile([128, 1024], CDT, tag="xt", name=f"xt1_{cb}")
            nc.sync.dma_start(out=x_t, in_=xT[cb, :, 1024:2048])
            xt_all[1].append(x_t)
        for i in range(8):
            nc.sync.dma_start(out=wproj_t8[i], in_=wproj[i])
        # ones columns of VV: constant, no DMA ring needed
        for gtb in range(16):
            nc.gpsimd.memset(VV[gtb][:, :, D:D + 1], 1.0)

        at_all, ags = {}, {}
        with tc.tile_critical():
            rid = nc.sync.partition_id()
            rankoff = (rid % 2) * (QC // 2)

        # ---- QKV phase units (injected into the attention k-tile loops)
        def qk_unit(tch, jt, s5):
            t0 = tch * 1024
            xt = xt_all[tch]
            ps = ps_s.tile([128, N5], F32, tag="s", name=f"pqk{tch}{jt}{s5}")
            for cb in range(8):
                nc.tensor.matmul(
                    ps, wqk_t[cb][:, jt * 128:(jt + 1) * 128],
                    xt[cb][:, s5 * N5:(s5 + 1) * N5],
                    start=(cb == 0), stop=(cb == 7))
            nc.vector.tensor_copy(
                qkT[jt][:, t0 + s5 * N5: t0 + (s5 + 1) * N5], ps)

        def v_unit(gtb):
            tch, tb = gtb // 8, gtb % 8
            xt = xt_all[tch]
            ps = ps_s.tile([128, CL], F32, tag="s", name=f"pv{gtb}")
            for cb in range(8):
                nc.tensor.matmul(
                    ps, xt[cb][:, tb * 128:(tb + 1) * 128], wv_t[cb],
                    start=(cb == 0), stop=(cb == 7))
            nc.vector.tensor_copy(
                VV[gtb][:, :, 0:D],
                ps.rearrange("p (h d) -> p h d", h=HL))

        def exchange_half(pqc, half):
            """AllGather heads half*4..half*4+3 (at tiles ci=2*half,2*half+1)."""
            pat = at_all[pqc]
            ad = drp.tile([256, QC], CDT, tag=f"atd{half}", name=f"atd{pqc}_{half}")
            for k in range(2):
                nc.sync.dma_start(
                    out=ad[k * 128:(k + 1) * 128, :], in_=pat[2 * half + k])
            ag = drp.tile([2, 256, QC], CDT, tag=f"atg{half}",
                          name=f"atg{pqc}_{half}")
            nc.gpsimd.collective_compute(
                "AllGather", mybir.AluOpType.bypass,
                replica_groups=[[0, 1], [2, 3], [4, 5], [6, 7]],
                ins=[ad[:]], outs=[ag[:]])
            ags[(pqc, half)] = ag

        # ci8 (128-row block of the 1024-ch at) -> (rank, half, k):
        # global channel block = rank*512 + half*256 + k*128
        def agt_fetch(pqc, halves=(0, 1)):
            agt = {}
            for ci8 in range(8):
                rank, half, k = ci8 // 4, (ci8 // 2) % 2, ci8 % 2
                if half not in halves:
                    continue
                t = stg.tile([128, QC // 2], CDT, tag=f"agt{ci8}",
                             name=f"agt{pqc}_{ci8}")
                nc.sync.dma_start(
                    out=t,
                    in_=ags[(pqc, half)][rank, k * 128:(k + 1) * 128,
                                         bass.ds(rankoff, QC // 2)])
                agt[ci8] = t
            return agt

        def proj_chain(pqc, agt, tt, jc, ci8s, start, stop, ps=None):
            if ps is None:
                ps = ps_s.tile([128, N5], F32, tag="s",
                               name=f"pp{pqc}_{tt}_{jc}")
            first = start
            for ci8 in ci8s:
                nc.tensor.matmul(
                    ps, agt[ci8][:, tt * 128:(tt + 1) * 128],
                    wproj_t8[ci8][:, jc * N5:(jc + 1) * N5],
                    start=first, stop=False)
                first = False
            if stop:
                nc.tensor.matmul(
                    ps, ones_t, bias_t[0:1, jc * N5:(jc + 1) * N5],
                    start=False, stop=True)
            return ps

        def proj_out(pqc, tt, st):
            r0 = pqc * (QC // 2) + tt * 128
            nc.sync.dma_start(out=out[r0:r0 + 128, 0:N5], in_=st[:, 0:N5])
            nc.sync.dma_start(out=out[r0:r0 + 128, N5:C], in_=st[:, N5:C])

        def proj_piece(pqc, agt):
            """proj + output for MY 256 rows (rank offset) of chunk pqc."""
            for tt in range(QC // 256):
                st = stg.tile([128, C], F32, tag="stage", name=f"stg{pqc}_{tt}")
                for jc in range(2):
                    ps = proj_chain(pqc, agt, tt, jc, range(8), True, True)
                    nc.vector.tensor_copy(st[:, jc * N5:(jc + 1) * N5], ps)
                proj_out(pqc, tt, st)

        def emit_attention(qc, fillers, cadence, boundary, post=None):
            """attention for chunk qc; `fillers` are injected into the k-tile
            loops every `cadence` iterations; `boundary[hp]` fns run after
            hp's loop."""
            q0 = qc * QC
            nkt = (q0 + QC) // KT
            at = [atp.tile([128, QC], CDT, tag=f"at{ci}", name=f"at{qc}_{ci}")
                  for ci in range(4)]
            at_all[qc] = at
            fl = list(fillers)
            for hp in range(HL // 2):
                heads = (2 * hp, 2 * hp + 1)
                av = {h: ps_av.tile([D + 1, N5], F32, tag="av",
                                    name=f"av{qc}_{h}")
                      for h in heads}
                exps = {}

                def emit_scores(kt):
                    k0 = kt * KT
                    est = max(0, k0 - q0)
                    # pair-shared score tile: head h at free half h%2
                    sp = ps_s.tile([128, 2, N5], F32, tag="s",
                                   name=f"s{qc}_{hp}_{kt}")
                    for h in heads:
                        roff = (h % 2) * D
                        nc.tensor.matmul(
                            sp[:, h % 2, est:N5],
                            qkT[4 + h // 2][roff:roff + D, k0:k0 + KT],
                            qkT[h // 2][roff:roff + D, q0 + est:q0 + QC],
                            start=True, stop=True)
                    ex = expp.tile([128, 2, N5], CDT, tag="exp",
                                   name=f"ex{qc}_{hp}_{kt}")
                    nc.scalar.activation(
                        ex[:, :, est:N5], sp[:, :, est:N5],
                        AF.Exp, scale=0.125)
                    if k0 >= q0:  # zero masked part of the diagonal slab
                        nc.vector.tensor_mul(
                            ex[:, :, est:est + KT],
                            ex[:, :, est:est + KT], keep2_t)
                    exps[kt] = ex

                def emit_attnv(kt):
                    k0 = kt * KT
                    cst = max(0, k0 - q0)
                    ex = exps.pop(kt)
                    for h in heads:
                        nc.tensor.matmul(
                            av[h][:, cst:N5], VV[kt][:, h, :],
                            ex[:, h % 2, cst:N5],
                            start=(kt == 0), stop=(kt == nkt - 1))

                emit_scores(0)
                for kt in range(1, nkt):
                    emit_scores(kt)
                    emit_attnv(kt - 1)
                    if kt % cadence == 0 and fl:
                        fl.pop(0)()
                emit_attnv(nkt - 1)

                for fn in boundary.get(hp, []):
                    fn()

                for h in heads:
                    roff = (h % 2) * D
                    a = av[h]
                    # custom-DVE/gpsimd ops need partition-0-aligned inputs;
                    # plain DVE copy handles the 64->0 shift (PSUM read)
                    rc0 = nrm.tile([1, N5], F32, tag="rc0",
                                   name=f"rc0{qc}_{h}")
                    nc.vector.tensor_copy(rc0, a[D:D + 1, :])
                    rc = nrm.tile([1, N5], F32, tag="rc", name=f"rc{qc}_{h}")
                    nc.vector.reciprocal_approx_fast(out=rc, in_=rc0)
                    rb = nrm.tile([D, N5], F32, tag="rb", name=f"rb{qc}_{h}")
                    nc.gpsimd.partition_broadcast(rb, rc)
                    nc.vector.tensor_mul(at[h // 2][roff:roff + D, :],
                                         a[0:D, :], rb)
                if hp == 1:
                    exchange_half(qc, 0)
                elif hp == 3:
                    exchange_half(qc, 1)
                for fn in (post or {}).get(hp, []):
                    fn()

        # ---- schedule -----------------------------------------------------
        def U(tch, jt, s5):
            return lambda: qk_unit(tch, jt, s5)

        def Vu(gtb):
            return lambda: v_unit(gtb)

        # bootstrap: exactly what qc0-hp0 needs (heads 0,1 + V tiles 0-3).
        # Pairs of units run cb-major so the PE gets two matmuls per input
        # DMA arrival instead of idling between them.
        def qk_pair(tch, jta, jtb, s5):
            t0 = tch * 1024
            xt = xt_all[tch]
            pa = ps_s.tile([128, N5], F32, tag="s", name=f"pqk{tch}{jta}{s5}")
            pb = ps_s.tile([128, N5], F32, tag="s", name=f"pqk{tch}{jtb}{s5}")
            for cb in range(8):
                for jt, ps in ((jta, pa), (jtb, pb)):
                    nc.tensor.matmul(
                        ps, wqk_t[cb][:, jt * 128:(jt + 1) * 128],
                        xt[cb][:, s5 * N5:(s5 + 1) * N5],
                        start=(cb == 0), stop=(cb == 7))
            for jt, ps in ((jta, pa), (jtb, pb)):
                nc.vector.tensor_copy(
                    qkT[jt][:, t0 + s5 * N5: t0 + (s5 + 1) * N5], ps)

        def v_pair(ga, gb):
            xt = xt_all[ga // 8]
            pa = ps_s.tile([128, CL], F32, tag="s", name=f"pv{ga}")
            pb = ps_s.tile([128, CL], F32, tag="s", name=f"pv{gb}")
            for cb in range(8):
                for gtb, ps in ((ga, pa), (gb, pb)):
                    tb = gtb % 8
                    nc.tensor.matmul(
                        ps, xt[cb][:, tb * 128:(tb + 1) * 128], wv_t[cb],
                        start=(cb == 0), stop=(cb == 7))
            for gtb, ps in ((ga, pa), (gb, pb)):
                nc.vector.tensor_copy(
                    VV[gtb][:, :, 0:D],
                    ps.rearrange("p (h d) -> p h d", h=HL))

        qk_pair(0, 0, 4, 0)
        v_pair(0, 1)
        v_pair(2, 3)

        agts = {}

        def PF(pqc, halves=(0, 1)):
            def f():
                agts.setdefault(pqc, {}).update(agt_fetch(pqc, halves))
            return f

        def PJ(pqc):
            return lambda: proj_piece(pqc, agts.pop(pqc))

        emit_attention(
            0,
            fillers=[Vu(4), Vu(5), Vu(6), Vu(7),
                     U(0, 0, 1), U(0, 4, 1), U(0, 1, 1), U(0, 5, 1),
                     U(0, 2, 1), U(0, 6, 1), U(0, 3, 1), U(0, 7, 1)],
            cadence=1,
            boundary={0: [U(0, 1, 0), U(0, 5, 0)],
                      1: [U(0, 2, 0), U(0, 6, 0)],
                      2: [U(0, 3, 0), U(0, 7, 0)]})
        emit_attention(
            1,
            fillers=[U(1, 0, 0), U(1, 4, 0), Vu(8),
                     U(1, 1, 0), U(1, 5, 0), Vu(9),
                     U(1, 2, 0), U(1, 6, 0), Vu(10),
                     U(1, 3, 0), U(1, 7, 0), Vu(11)],
            cadence=2,
            boundary={2: [PF(0)]})
        emit_attention(
            2,
            fillers=[U(1, 0, 1), U(1, 4, 1), Vu(12),
                     U(1, 1, 1), U(1, 5, 1), Vu(13),
                     U(1, 2, 1), U(1, 6, 1), Vu(14),
                     U(1, 3, 1), U(1, 7, 1), Vu(15)],
            cadence=3,
            boundary={1: [PF(1)]})

        # last chunk: proj3's first-half-channel matmuls (ci8 0,1,4,5 from
        # the hp0/hp1 exchange) run at the hp3 boundary so the PE has work
        # while the final AllGather (heads 4-7) is in flight.
        p3 = {}

        def proj3_partA():
            p3['st'] = [stg.tile([128, C], F32, tag="stage", name=f"stg3_{tt}")
                        for tt in range(2)]
            p3['ps'] = [proj_chain(3, agts[3], 0, jc, (0, 1, 4, 5),
                                   True, False) for jc in range(2)]

        emit_attention(
            3,
            fillers=[], cadence=100,
            boundary={0: [PJ(0)], 1: [PJ(1)], 2: [PF(2), PF(3, halves=(0,))]},
            post={3: [PJ(2), proj3_partA]})
        agts[3].update(agt_fetch(3, halves=(1,)))
        st = p3['st']
        for jc in range(2):
            ps = proj_chain(3, agts[3], 0, jc, (2, 3, 6, 7),
                            False, True, ps=p3['ps'][jc])
            nc.vector.tensor_copy(st[0][:, jc * N5:(jc + 1) * N5], ps)
        proj_out(3, 0, st[0])
        for jc in range(2):
            ps = proj_chain(3, agts[3], 1, jc, range(8), True, True)
            nc.vector.tensor_copy(st[1][:, jc * N5:(jc + 1) * N5], ps)
        proj_out(3, 1, st[1])


def _prepare_in_maps(x, Wqkv, Wproj, bproj):
    x = np.asarray(x, dtype=np.float32)
    Wqkv = np.asarray(Wqkv, dtype=np.float32)
    Wproj = np.asarray(Wproj, dtype=np.float32)
    bproj = np.asarray(bproj, dtype=np.float32)

    # triangular keep-mask slab: 1 where q >= k (kept), 0 where masked;
    # duplicated side by side for the two heads of a pair
    k_i = np.arange(128)[:, None]
    q_i = np.arange(128)[None, :]
    keep = np.where(q_i >= k_i, np.float32(1.0), np.float32(0.0))
    keep2 = np.ascontiguousarray(
        np.concatenate([keep, keep], axis=1), dtype=np.float32)

    ones_r = np.ones((1, 128), dtype=np.float32)

    in_maps = []
    for core in range(8):
        b, hg = core // 2, core % 2
        xT = np.ascontiguousarray(x[b].T).reshape(8, 128, T)
        wq = Wqkv[:, hg * CL:(hg + 1) * CL]
        wk = Wqkv[:, C + hg * CL: C + (hg + 1) * CL]
        wv_ = Wqkv[:, 2 * C + hg * CL: 2 * C + (hg + 1) * CL]
        wqk = np.ascontiguousarray(
            np.concatenate([wq, wk], axis=1)).reshape(8, 128, 1024)
        wv = np.ascontiguousarray(wv_).reshape(8, 128, CL)
        wp = np.ascontiguousarray(Wproj).reshape(8, 128, C)
        bf = ml_dtypes.bfloat16
        in_maps.append({
            "xT": xT.astype(bf), "wqk": wqk.astype(bf), "wv": wv.astype(bf),
            "wproj": wp.astype(bf),
            "bias2": bproj.reshape(1, C).astype(bf),
            "ones_r": ones_r.astype(bf),
            "keep2": keep2.astype(bf),
        })
    return in_maps


def _assemble(results):
    full = np.empty((B, T, C), dtype=np.float32)
    for core in range(8):
        b, r = core // 2, core % 2
        o = results[core]["out"]  # [1024, 1024]
        for qc in range(NQC):
            g0 = qc * QC + r * (QC // 2)
            full[b, g0:g0 + QC // 2] = \
                o[qc * (QC // 2):(qc + 1) * (QC // 2)]
    return full


_NC_CACHE = None


def kernel(x, Wqkv, Wproj, bproj):
    global _NC_CACHE
    if _NC_CACHE is None:
        _NC_CACHE = _build()
    in_maps = _prepare_in_maps(x, Wqkv, Wproj, bproj)
    res = run_bass_kernel_spmd(_NC_CACHE, in_maps, list(range(8)))
    return _assemble(res.results)
